# revision 12
# baseline (speedup 1.0000x reference)
"""Trainium2 Bass kernel: depthwise (per-sample, per-channel) 15x15 'same'
true convolution of 1024x3 images of 128x128, data-parallel over 8 NeuronCores.

Formulation (per (bn,c) pair, P=128, K=15, pad=7):
    out[y,x] = sum_{dy,dx} Xp[y+dy, x+dx] * Wf[dy,dx],   Wf = flip(kernel),
    Xp = zero-padded image [142, 143].
Output rows split into 4 blocks of 32 (j in 0..31); block b runs on the 32-wide
column strip 32b of the PE array. The 15 dx passes ping-pong between the two
64-row halves of the array (even dx -> rows 0..45, odd dx -> rows 64..109,
tile_position=(64*(dx%2), 32*b)) so each strip's next LDWEIGHTS targets the
idle row half and pulls ahead of the in-flight matmul — avoiding the
LDWEIGHTS/MATMUL serialization that otherwise dominates (weights change every
matmul here). Per pass: stationary Toeplitz slab T[i, j] = Wf[i-j, dx]
([46, 32]), moving = image window rows 32b..32b+45 with column offset dx.
Even passes accumulate in PSUM tile A, odd in tile B (row tiles must not share
a PSUM bank); DVE adds A+B and casts to fp16 at evacuation.

Data staging (per group of G=32 pairs, all via gpsimd SWDGE, which stripes
descriptors across DMA engines 2..15 — the two HWDGE rings are pinned to
engines 0/1): images stored pair-interleaved in DRAM ([row][pair][143]) so
each block's window tile ([128, G*143], window duplicated at partitions 0..45
and 64..109) loads with 9152-byte runs; Toeplitz slabs and fp16 outputs are
group-batched. Sharding: pure data parallel over BN (128 samples x 3 channels
= 384 pairs per core).
"""
import sys

sys.path.insert(0, "/opt/trn_rl_repo")

import numpy as np

_N_CORES = 8
_BN, _C, _P, _K = 1024, 3, 128, 15
_PAIRS_PER_CORE = (_BN // _N_CORES) * _C  # 384
_G = 32                      # pairs per DMA group
_NG = _PAIRS_PER_CORE // _G  # 12
_XW = 143                    # padded image width (cols 0..142)
_XH = 142                    # padded image height
_ROWP = _G * _XW             # elems per padded row across a group (4576)
_GRP = _XH * _ROWP           # elems per group image block
_TSLAB = _G * 8 * 32         # T free elems per partition per group

_nc_cache = {}


def _build_nc(bufs: int = 2, psum_bufs: int = 4, dup_dma: bool = False):
    import concourse.bacc as bacc
    import concourse.mybir as mybir
    from concourse import bass, tile

    FP16 = mybir.dt.float16
    FP32 = mybir.dt.float32

    nc = bacc.Bacc("TRN2", target_bir_lowering=False, debug=False)
    xpad_d = nc.dram_tensor("xpad", [_NG * _GRP + 64], FP16, kind="ExternalInput")
    toep_d = nc.dram_tensor("toep", [_NG, 2, 46, _TSLAB], FP16, kind="ExternalInput")
    out_d = nc.dram_tensor("out", [_NG, 128, _G * 128], FP16, kind="ExternalOutput")
    xt = xpad_d.tensor if hasattr(xpad_d, "tensor") else xpad_d

    with tile.TileContext(nc) as tc:
        with (
            tc.tile_pool(name="xb", bufs=bufs) as xb_pool,
            tc.tile_pool(name="tt", bufs=bufs) as tt_pool,
            tc.tile_pool(name="ot", bufs=bufs) as ot_pool,
            tc.tile_pool(name="tmp", bufs=4) as tmp_pool,
            tc.tile_pool(name="ps", bufs=psum_bufs, space="PSUM") as ps_pool,
        ):
            for grp in range(_NG):
                xb = [xb_pool.tile([128, _ROWP], FP16, tag=f"xb{b}",
                                   name=f"xb{b}")
                      for b in range(4)]
                tt = tt_pool.tile([128, _TSLAB], FP16, tag="tt")
                ot = ot_pool.tile([128, _G * 128], FP16, tag="ot")

                for b in range(4):
                    # window rows 32b..32b+45 (pair-interleaved), duplicated
                    # at partitions 0..45 and 64..109
                    src = bass.AP(
                        tensor=xt,
                        offset=grp * _GRP + 32 * b * _ROWP,
                        ap=[[0, 2], [_ROWP, 46], [1, _ROWP]],
                    )
                    if dup_dma:
                        xap = xb[b][:]
                        dst = bass.AP(
                            tensor=xap.tensor,
                            offset=xap.offset,
                            ap=[[64 * _ROWP, 2], [_ROWP, 46], [1, _ROWP]],
                        )
                        nc.gpsimd.dma_start(out=dst, in_=src)
                    else:
                        src1 = bass.AP(
                            tensor=xt,
                            offset=grp * _GRP + 32 * b * _ROWP,
                            ap=[[_ROWP, 46], [1, _ROWP]],
                        )
                        nc.gpsimd.dma_start(out=xb[b][0:46, :], in_=src1)
                        nc.gpsimd.dma_start(out=xb[b][64:110, :], in_=src1)
                nc.gpsimd.dma_start(out=tt[0:46, :], in_=toep_d[grp, 0])
                nc.gpsimd.dma_start(out=tt[64:110, :], in_=toep_d[grp, 1])

                for g in range(_G):
                    psA = ps_pool.tile([128, 128], FP32, tag="psA")
                    psB = ps_pool.tile([128, 128], FP32, tag="psB")
                    ps = (psA, psB)
                    for dx in range(15):
                        p = dx & 1
                        slot = dx >> 1
                        for b in range(4):
                            nc.tensor.matmul(
                                ps[p][32 * b:32 * b + 32, :],
                                tt[64 * p:64 * p + 46,
                                   (g * 8 + slot) * 32:(g * 8 + slot) * 32 + 32],
                                xb[b][64 * p:64 * p + 46,
                                      g * _XW + dx:g * _XW + dx + 128],
                                start=(dx < 2), stop=(dx >= 13),
                                tile_position=(64 * p, 32 * b),
                            )
                    tmp = tmp_pool.tile([128, 128], FP32, tag="tmp")
                    nc.scalar.copy(tmp[:], psA[:])
                    nc.vector.tensor_add(
                        ot[:, g * 128:(g + 1) * 128], tmp[:], psB[:])

                nc.gpsimd.dma_start(out=out_d[grp], in_=ot[:])

    nc.compile()
    return nc


def _host_prep(patches_pairs: np.ndarray, kernels_pairs: np.ndarray):
    """[NP,128,128] f32, [NP,15,15] f32 -> (xpad flat fp16, toep fp16).

    xpad: [NG*142*G*143 + 64] with layout [grp][row 142][pair G][col 143],
    zero-padded images at rows/cols 7..134.
    toep: [NG, 2, 46, G, 8, 32]: parity 0 slots hold dx=2e, parity 1 slots
    dx=2o+1, T[i, slot, j] = Wf[i-j, dx] for 0 <= i-j < 15.
    """
    NP = patches_pairs.shape[0]
    assert NP == _PAIRS_PER_CORE
    Xp = np.zeros((_NG, _G, _XH, _XW), dtype=np.float16)
    Xp[:, :, 7:135, 7:135] = patches_pairs.reshape(_NG, _G, 128, 128)
    xpad = np.zeros(_NG * _GRP + 64, dtype=np.float16)
    xpad[:_NG * _GRP] = np.ascontiguousarray(
        Xp.transpose(0, 2, 1, 3)).reshape(-1)

    Wf = np.ascontiguousarray(
        kernels_pairs[:, ::-1, ::-1]).astype(np.float16)  # [NP, 15, 15]
    T = np.zeros((NP, 2, 46, 8, 32), dtype=np.float16)
    j = np.arange(32)
    for dy in range(15):
        for slot in range(8):
            for par in range(2):
                dx = 2 * slot + par
                if dx > 14:
                    continue
                T[:, par, j + dy, slot, j] = Wf[:, dy, dx][:, None]
    T = T.reshape(_NG, _G, 2, 46, 8 * 32).transpose(0, 2, 3, 1, 4)
    toep = np.ascontiguousarray(T).reshape(_NG, 2, 46, _TSLAB)
    return xpad, toep


def kernel(patches, kernels, kernel_size, patch_size, fft_size, _collect_results=None):
    """Full inputs in, full output out. Shards BN across 8 cores."""
    from concourse.bass_utils import run_bass_kernel_spmd

    patches = np.asarray(patches)
    kernels = np.asarray(kernels)
    assert patches.shape == (_BN, _C, _P, _P), patches.shape
    assert kernels.shape == (_BN, _C, _K, _K), kernels.shape

    if "nc" not in _nc_cache:
        _nc_cache["nc"] = _build_nc()
    nc = _nc_cache["nc"]

    bn_per_core = _BN // _N_CORES
    in_maps = []
    for core in range(_N_CORES):
        sl = slice(core * bn_per_core, (core + 1) * bn_per_core)
        pp = patches[sl].reshape(-1, _P, _P)
        kp = kernels[sl].reshape(-1, _K, _K)
        xpad, toep = _host_prep(pp, kp)
        in_maps.append({"xpad": xpad, "toep": toep})

    res = run_bass_kernel_spmd(nc, in_maps, core_ids=list(range(_N_CORES)))
    if _collect_results is not None:
        _collect_results.append(res)

    out = np.empty((_BN, _C, _P, _P), dtype=np.float32)
    for core in range(_N_CORES):
        sl = slice(core * bn_per_core, (core + 1) * bn_per_core)
        o = res.results[core]["out"].reshape(_NG, 128, _G, 128)
        out[sl] = o.transpose(0, 2, 1, 3).reshape(
            bn_per_core, _C, _P, _P).astype(np.float32)
    return out
